# revision 3
# baseline (speedup 1.0000x reference)
"""Trainium2 Bass kernel for nn_CTAModule (pooled-token attention over video).

Computation (per (b,c) head, t=16 tokens):
  pooled = AvgPool7x7(x)                  (t, 8, 8) -> tokens (t, 64)
  s      = LN(pooled + pos) @ W_qk        -> q, k  (t, 64) each
  attn   = softmax(q @ k^T / 8)           (t, t)
  out    = attn @ v + x,   v = x rows     (t, 3136)

Sharding: pure data-parallel over the fused (b*c)=512 head axis; core i
takes b==i (64 heads). Per core, heads are processed in 8 groups of 8
heads = 128 partition rows (t-major: p = t*8 + c_local).

Design (memory-bound; measured DMA floor ~30us/iter at 2B/elem):
  - x and out cross HBM as bf16 (host converts): 32.5MB -> 16.3MB per
    core, halving the f32 DMA roofline. All accumulation stays f32;
    measured end-to-end rel err ~3e-3 vs the f32 reference (gate 2e-2).
  - residual + softmax normalization folded into the PE: the 16x16
    attention matrix is scaled by 1/denom (one tiny DVE op) and the
    transpose PSUM accumulates ident.T@ident=I, so attn@v computes
    (A+I)@x in one pass; the big-tile epilogue is a pure PSUM->SBUF
    bf16 cast copy, split ACT(2048)/DVE(1088) per group.
  - cross-head mask (-1e30 stripe) folded into the dots PSUM via a
    second accumulating matmul; exp reads PSUM directly; no max-sub
    (logits are bounded), killing the DVE mask/rowmax pass.
  - q/k biases ride row 64 of W via a fixed ones-column in sln, so
    q,k land in one PSUM tile with one 256-wide copy-out and no
    bias pass.
  - 7x7 mean pool split by output rows: GpSimd tree-sums the w-window
    for hp rows 0-4 (6 two-input adds, depth 3); DVE reduces rows 5-7
    straight from x with a 4-D (dh,dw) XY-reduce. Keeping both engines
    on pool beats consolidating on the faster DVE (HW-measured).
  - rsqrt(var+eps) by Newton-Raphson on DVE (bit-trick seed on
    (var+eps)/2 with magic 0x5EF359DF, two fused 2-op iterations) -
    avoids ACT table-set thrash with the Exp set.
  - scheduling for the Tile framework's in-order engine queues:
    chains of a pair of groups are interleaved op-by-op (hides
    dependent-op latency); each pair's AV/epilogue blocks are emitted
    TWO pairs behind the chains (stagger depth 2 - worth ~7% on HW
    where semaphore latency exceeds the cost model); X tiles
    triple-buffered so next-iteration loads prefetch; the ACT-owned
    first half of each Y stores early, the DVE-owned tail store is
    deferred one group (both dispatched by ACT, never stalling on a
    cross-engine copy).
"""

import numpy as np

B, T, C, H, W = 8, 16, 64, 56, 56
HW = H * W            # 3136
DIM = 8               # pooled spatial
PH = H // DIM         # 7
NGRP = 8              # groups per core (8 heads each)
GP = 128              # partitions per group = 8 heads * 16 t
LN_EPS = 1e-5
SCALE = 64 ** -0.5    # dim_head^-0.5 = 0.125
NCORES = 8
POOL_SPLIT = 280      # pool1 columns on GpSimd (tree); rest on DVE reduce
# Y chunking: (size, engine) — 2-bank PSUM tiles, copies spread ACT/DVE;
# ACT owns [0:2048] and stores that half, DVE owns the tail and stores it
Y_CHUNKS = [(1024, "act"), (1024, "act"), (1024, "vector"), (64, "vector")]

_CACHE = {}


def _build_nc(repeat=1, bench=False):
    import concourse.bass as bass  # noqa: F401
    import concourse.bacc as bacc
    import concourse.tile as tile
    import concourse.mybir as mybir

    dt = mybir.dt
    F = mybir.ActivationFunctionType
    ALU = mybir.AluOpType
    AX = mybir.AxisListType

    nc = bacc.Bacc("TRN2", target_bir_lowering=False, debug=False,
                   num_devices=NCORES)

    # bench mode: big tensors become device-internal scratch (zeroed
    # in-kernel) so repeated timed executions don't move 200MB over the
    # axon tunnel; tiny token tensors keep the executable shape stable.
    big_kind = "Internal" if bench else None
    xs = nc.dram_tensor("xs", (T, C, HW), dt.bfloat16,
                        kind=big_kind or "ExternalInput")
    # host supplies pos pre-arranged as (t, c_local, g*64+f) so one DMA
    # covers all 8 groups with a contiguous free dim
    pos = nc.dram_tensor("pos", (T, 8, NGRP * DIM * DIM), dt.float32,
                         kind=big_kind or "ExternalInput")
    # row 64 holds beta@W_qk; sln's 65th ones-column applies it in the
    # q/k matmul itself (no separate bias pass)
    w = nc.dram_tensor("w", (DIM * DIM + 1, 128), dt.float32,
                       kind="ExternalInput")
    out = nc.dram_tensor("out", (T, C, HW), dt.bfloat16,
                         kind=big_kind or "ExternalOutput")
    if bench:
        tok_out = nc.dram_tensor("tok_out", (1, 16), dt.float32,
                                 kind="ExternalOutput")

    ident_dram = nc.inline_tensor(np.eye(128, dtype=np.float32), name="ident")
    # rows are t-major (p = t*8 + c_local): same-head pairs are p%8 == f%8
    pp, ff = np.meshgrid(np.arange(128), np.arange(128), indexing="ij")
    mask_np = np.where(pp % 8 == ff % 8, 0.0, -1e30).astype(np.float32)
    mask_dram = nc.inline_tensor(mask_np, name="attn_mask")

    G = NGRP

    with tile.TileContext(nc) as tc:
        with (
            tc.tile_pool(name="cp", bufs=1) as cp,
            tc.tile_pool(name="xp", bufs=3) as xp,
            tc.tile_pool(name="yp", bufs=3) as yp,
            tc.tile_pool(name="sp", bufs=1) as sp,
            tc.tile_pool(name="wp", bufs=2) as wp,
            tc.tile_pool(name="pvp", bufs=2, space="PSUM") as pvp,
            tc.tile_pool(name="psp", bufs=4, space="PSUM") as psp,
        ):
            # ---- constants (loaded once) ----
            w_sb = cp.tile([65, 128], dt.float32)
            nc.sync.dma_start(w_sb[:], w[:])
            ident_sb = cp.tile([128, 128], dt.float32)
            nc.sync.dma_start(ident_sb[:], ident_dram[:])
            mask_sb = cp.tile([128, 128], dt.float32)
            nc.sync.dma_start(mask_sb[:], mask_dram[:])
            c1p5_sb = cp.tile([128, 1], dt.float32)
            nc.vector.memset(c1p5_sb[:], 1.5)
            magic_sb = cp.tile([128, 1], dt.uint32)
            nc.vector.memset(magic_sb[:], 0x5F3759DF - 0x00400000)
            # fixed sln tiles (even/odd groups) with the ones-column that
            # routes W's bias row — initialized once, not per group
            sln_ab = []
            for j in range(2):
                t = cp.tile([GP, 65], dt.float32, tag=f"sln{j}",
                            name=f"sln{j}")
                nc.vector.memset(t[:, 64:65], 1.0)
                sln_ab.append(t)
            if bench:
                # zero the scratch inputs so compute never sees NaNs;
                # borrow rotating pool tiles so this costs no extra SBUF
                zt = yp.tile([GP, HW], dt.bfloat16, tag="Y", name="zt")
                nc.vector.memset(zt[:], 0.0)
                zp = sp.tile([GP, NGRP * 64], dt.float32, tag="Pall",
                             name="zp", bufs=2)
                nc.vector.memset(zp[:], 0.0)
                nc.sync.dma_start(pos[:], zp[:])
                for g in range(NGRP):
                    nc.sync.dma_start(xs[:, 8 * g:8 * g + 8, :], zt[:])
                tk = cp.tile([1, 16], dt.float32)
                nc.vector.memset(tk[:], 0.0)
                nc.sync.dma_start(tok_out[:], tk[:])

            for _ in range(repeat):
                # ---- phase 1: load all groups (dst partition runs pair
                # in order with (t, c) src runs -> p = t*8 + c_local) ----
                # all pos embeddings in one DMA: dst [p, g*64+f]
                Pall = sp.tile([GP, G * 64], dt.float32, tag="Pall",
                               name="Pall", bufs=2)
                nc.sync.dma_start(Pall[:], pos[:])
                Xs = []
                for g in range(G):
                    c0 = 8 * g
                    X = xp.tile([GP, HW], dt.bfloat16, tag=f"X{g}",
                                name=f"X{g}")
                    nc.sync.dma_start(X[:], xs[:, c0:c0 + 8, :])
                    Xs.append(X)

                # ---- software-pipelined compute: chains of a PAIR of
                # groups are interleaved op-by-op (hides dependent-op
                # latency on DVE/ACT/PE), and each pair's epilogue blocks
                # are emitted behind the next pair's chains ----
                bdTs = {}

                def chain_steps(g):
                    X = Xs[g]
                    Xw = X[:].rearrange("p (a dw) -> p a dw",
                                        a=H * DIM, dw=PH)
                    # pool split by output rows: GpSimd tree-sums the
                    # w-window for hp rows [0, HP_G) into s1; DVE reduces
                    # hp rows [HP_G, 8) straight from x into pooled via a
                    # 4-D (dh, dw) XY-reduce. 56 columns per hp row.
                    HP_G = POOL_SPLIT // 56
                    SP_ = HP_G * 56
                    s1 = sp.tile([GP, SP_], dt.float32, tag=f"s1{g}",
                                 name=f"s1{g}")
                    Xg = Xw[:, 0:SP_, :]
                    ta = wp.tile([GP, SP_], dt.float32, tag="poolA")
                    tb = wp.tile([GP, SP_], dt.float32, tag="poolB")
                    nc.gpsimd.tensor_add(s1[:], Xg[:, :, 0], Xg[:, :, 1])
                    yield
                    nc.gpsimd.tensor_add(ta[:], Xg[:, :, 2], Xg[:, :, 3])
                    yield
                    nc.gpsimd.tensor_add(tb[:], Xg[:, :, 4], Xg[:, :, 5])
                    yield
                    nc.gpsimd.tensor_add(s1[:], s1[:], ta[:])
                    yield
                    nc.gpsimd.tensor_add(tb[:], tb[:], Xg[:, :, 6])
                    yield
                    nc.gpsimd.tensor_add(s1[:], s1[:], tb[:])
                    yield

                    pooled = wp.tile([GP, 64], dt.float32, tag="pooled")
                    nc.vector.reduce_sum(
                        pooled[:, 0:HP_G * DIM],
                        s1[:].rearrange("p (hp dh w) -> p hp w dh",
                                        hp=HP_G, dh=PH, w=DIM),
                        axis=AX.X)
                    yield
                    X5 = X[:].rearrange(
                        "p (hp dh wp dw) -> p hp wp dh dw",
                        hp=DIM, dh=PH, wp=DIM, dw=PH)
                    nc.vector.reduce_sum(pooled[:, HP_G * DIM:],
                                         X5[:, HP_G:, :, :, :],
                                         axis=AX.XY)
                    yield
                    s = wp.tile([GP, 64], dt.float32, tag="s")
                    nc.vector.scalar_tensor_tensor(
                        s[:], pooled[:], 1.0 / (PH * PH),
                        Pall[:, 64 * g:64 * g + 64],
                        op0=ALU.mult, op1=ALU.add)
                    yield
                    st6 = wp.tile([GP, 6], dt.float32, tag="st6")
                    nc.vector.bn_stats(st6[:], s[:])
                    yield
                    st2 = wp.tile([GP, 2], dt.float32, tag="st2")
                    nc.vector.bn_aggr(st2[:], st6[:])
                    yield
                    # halfx = (var+eps)/2; NR seed uses magic tuned for
                    # halfx bits (0x5F3759DF - 0x00400000)
                    halfx = wp.tile([GP, 1], dt.float32, tag="halfx")
                    nc.vector.tensor_scalar(halfx[:], st2[:, 1:2], LN_EPS,
                                            0.5, op0=ALU.add, op1=ALU.mult)
                    yield
                    nhalfx = wp.tile([GP, 1], dt.float32, tag="nhalfx")
                    nc.vector.tensor_scalar_mul(nhalfx[:], halfx[:], -1.0)
                    yield
                    yb = wp.tile([GP, 1], dt.uint32, tag="yb")
                    nc.vector.tensor_scalar(yb[:],
                                            halfx[:].bitcast(dt.uint32),
                                            1, None,
                                            op0=ALU.arith_shift_right)
                    yield
                    nc.vector.tensor_tensor(yb[:], magic_sb[:], yb[:],
                                            op=ALU.subtract)
                    yield
                    y = yb[:].bitcast(dt.float32)
                    yy = wp.tile([GP, 1], dt.float32, tag="yy")
                    for _i in range(2):  # even # of NR iters -> positive
                        # w = (y * -halfx) * y;  y = (w + 1.5) * y
                        nc.vector.scalar_tensor_tensor(
                            yy[:], y, nhalfx[:], y,
                            op0=ALU.mult, op1=ALU.mult)
                        yield
                        nc.vector.scalar_tensor_tensor(
                            y, yy[:], 1.5, y, op0=ALU.add, op1=ALU.mult)
                        yield
                    sln = sln_ab[g % 2]
                    nc.vector.tensor_scalar(sln[:, 0:64], s[:],
                                            st2[:, 0:1], y,
                                            op0=ALU.subtract, op1=ALU.mult)
                    yield

                    sT_ps = psp.tile([65, 128], dt.float32, tag="smallps")
                    nc.tensor.transpose(sT_ps[:], sln[:], ident_sb[:])
                    yield
                    sT_sb = wp.tile([65, 128], dt.float32, tag="sT")
                    nc.scalar.copy(sT_sb[:], sT_ps[:])
                    yield

                    # q and k in one PSUM tile, one 256-wide copy out
                    qk_ps = psp.tile([64, 256], dt.float32, tag="smallps")
                    nc.tensor.matmul(qk_ps[:, 0:128], w_sb[:, 0:64],
                                     sT_sb[:])
                    yield
                    nc.tensor.matmul(qk_ps[:, 128:256], w_sb[:, 64:128],
                                     sT_sb[:])
                    yield
                    qk_sb = wp.tile([64, 256], dt.float32, tag="qk")
                    nc.scalar.copy(qk_sb[:], qk_ps[:])
                    yield

                    # dots + cross-head mask, both accumulated in PSUM
                    dots_ps = psp.tile([GP, 128], dt.float32, tag="smallps")
                    nc.tensor.matmul(dots_ps[:], qk_sb[:, 0:128],
                                     qk_sb[:, 128:256],
                                     start=True, stop=False)
                    nc.tensor.matmul(dots_ps[:], ident_sb[:], mask_sb[:],
                                     start=False, stop=True,
                                     skip_group_check=True)
                    yield

                    # exp straight off PSUM (logits bounded; no max-sub)
                    bd = wp.tile([GP, 128], dt.float32, tag="bd")
                    denom = wp.tile([GP, 1], dt.float32, tag="denom")
                    nc.scalar.activation(bd[:], dots_ps[:], F.Exp,
                                         scale=SCALE, accum_out=denom[:])
                    yield
                    recip = wp.tile([GP, 1], dt.float32, tag="recip")
                    nc.vector.reciprocal(recip[:], denom[:])
                    yield
                    # normalize the 16x16 attention matrix itself
                    nc.vector.tensor_scalar_mul(bd[:], bd[:], recip[:])
                    yield

                    # transpose + I: bdT_ps = bd.T + ident (residual fold)
                    bdT_ps = psp.tile([GP, 128], dt.float32, tag="smallps")
                    nc.tensor.matmul(bdT_ps[:], bd[:], ident_sb[:],
                                     is_transpose=True,
                                     start=True, stop=False)
                    nc.tensor.matmul(bdT_ps[:], ident_sb[:], ident_sb[:],
                                     start=False, stop=True,
                                     skip_group_check=True)
                    yield
                    bdT_sb = wp.tile([GP, 128], dt.bfloat16, tag="bdT",
                                     bufs=6)
                    nc.scalar.copy(bdT_sb[:], bdT_ps[:])
                    bdTs[g] = bdT_sb

                def chain2(ga, gb):
                    its = [chain_steps(ga), chain_steps(gb)]
                    live = [True, True]
                    while any(live):
                        for i, it in enumerate(its):
                            if live[i]:
                                try:
                                    next(it)
                                except StopIteration:
                                    live[i] = False

                pending_tail = []

                def flush_tail():
                    while pending_tail:
                        pc0, pY = pending_tail.pop(0)
                        nc.scalar.dma_start(out[:, pc0:pc0 + 8, 2048:],
                                            pY[:, 2048:])

                def avblock(g, last=False):
                    X = Xs[g]
                    bdT_sb = bdTs.pop(g)
                    c0 = 8 * g
                    # tail store of the previous group: its DVE chunks are
                    # long done, so ACT dispatches without stalling
                    flush_tail()
                    Y = yp.tile([GP, HW], dt.bfloat16, tag="Y")
                    n0 = 0
                    for nn, eng in Y_CHUNKS:
                        av = pvp.tile([GP, 1024], dt.float32, tag="av")
                        for m0 in range(0, nn, 512):
                            mm = min(nn - m0, 512)
                            nc.tensor.matmul(
                                av[:, m0:m0 + mm], bdT_sb[:],
                                X[:, n0 + m0:n0 + m0 + mm])
                        if eng == "act":
                            nc.scalar.copy(Y[:, n0:n0 + nn], av[:, :nn])
                        else:
                            nc.vector.tensor_copy(Y[:, n0:n0 + nn],
                                                  av[:, :nn])
                        n0 += nn
                        if n0 == 2048:
                            # ACT-owned half flies early, dispatched by
                            # ACT itself (no cross-engine queue stall)
                            nc.scalar.dma_start(
                                out[:, c0:c0 + 8, 0:2048], Y[:, 0:2048])
                    pending_tail.append((c0, Y))
                    if last:
                        flush_tail()

                pairs = [(0, 1), (2, 3), (4, 5), (6, 7)]
                for pi, (ga, gb) in enumerate(pairs):
                    chain2(ga, gb)
                    if pi >= 2:
                        avblock(pairs[pi - 2][0])
                        avblock(pairs[pi - 2][1])
                for pp in pairs[-2:]:
                    avblock(pp[0])
                    avblock(pp[1], last=(pp is pairs[-1]))

    nc.compile()
    return nc


def _get_nc(repeat=1):
    if repeat not in _CACHE:
        _CACHE[repeat] = _build_nc(repeat)
    return _CACHE[repeat]


def _make_in_maps(x, pos_embedding, W_qk, gamma, beta):
    import ml_dtypes
    bf16 = ml_dtypes.bfloat16
    x = np.asarray(x, dtype=np.float32)
    W_eff = np.concatenate(
        [gamma[:, None] * W_qk, (beta @ W_qk)[None, :]], axis=0)
    W_eff = np.ascontiguousarray(W_eff, dtype=np.float32)  # (65, 128)
    in_maps = []
    for i in range(NCORES):
        # shard (c=8g+cl, t, f) -> kernel layout (t, cl, g*64+f)
        pe = np.asarray(pos_embedding[i * C:(i + 1) * C],
                        dtype=np.float32).reshape(NGRP, 8, T, DIM * DIM)
        in_maps.append({
            "xs": np.ascontiguousarray(
                x[i].reshape(T, C, HW).astype(bf16)),
            "pos": np.ascontiguousarray(
                pe.transpose(2, 1, 0, 3).reshape(T, 8, NGRP * DIM * DIM)),
            "w": W_eff,
        })
    return in_maps


def kernel(x, pos_embedding, W_qk, gamma, beta, _repeat=1):
    from concourse import bass_utils
    nc = _get_nc(_repeat)
    in_maps = _make_in_maps(x, pos_embedding, W_qk, gamma, beta)
    res = bass_utils.run_bass_kernel_spmd(nc, in_maps,
                                          core_ids=list(range(NCORES)))
    outs = [np.asarray(r["out"], dtype=np.float32).reshape(T, C, H, W)
            for r in res.results]
    return np.stack(outs)
